# revision 1
# baseline (speedup 1.0000x reference)
"""Trainium2 Bass kernel for nn_MetricPoseLoss (RANSAC pose loss).

Data-parallel over batch B=8: one NeuronCore per batch sample; each core
handles IT_M*IT_R = 256 RANSAC hypotheses as 2 partition-tiles of 128
(hypothesis rows on SBUF partitions, S=256 sample points on the free axis).

Keypoint gather (idx = 128*c + r) runs on the PE: ones-matmul broadcasts
the r/c index rows, a one-hot (r==p) matmul pulls all 8 chunk candidates
per sample from the SBUF table, and a (c==p//4)-masked select matmul picks
the right chunk. Procrustes (weighted Kabsch) avoids SVD via the Horn
quaternion method: per-hypothesis weighted moment sums are PE matmuls of
transposed per-m integrand tables against the transposed weights; the max
eigenvalue of the 4x4 quaternion matrix comes from Newton iteration on its
characteristic quartic, the eigenvector from two adjugate-column
candidates.  All instructions avoid tensor_tensor_reduce accum_out and
GPSIMD ucode, which wedge the exec unit on this hardware.

kernel() caches the compiled jax/PJRT executable across calls; per-call
wall time is dominated by the axon-tunnel execute RPC (~40 ms).
"""
import os
import sys
import numpy as np
from contextlib import ExitStack

sys.path.insert(0, "/opt/trn_rl_repo")

import concourse.bass as bass  # noqa: E402
import concourse.bacc as bacc  # noqa: E402
import concourse.mybir as mybir  # noqa: E402
from concourse import bass_isa  # noqa: E402
from concourse.tile import TileContext  # noqa: E402

B = 8; NK = 1024; S = 256; IT_M = 16; IT_R = 16; NCON = 5
TH = 0.15; TEMP = 10.0; NREF = 4; BETA = 5.0
MAX_ROT = 45.0; MAX_TRANS = 1.0
NEWTON = 6
STAGE = int(os.environ.get("KERNSTAGE", "6"))
NTILE = 2  # 256 hyp rows per core = 2 x 128 partitions

F32 = mybir.dt.float32
F16 = mybir.dt.float16
I32 = mybir.dt.int32
I16 = mybir.dt.int16
OP = mybir.AluOpType
AF = mybir.ActivationFunctionType
AX = mybir.AxisListType
PI = float(np.pi)

ACOS_C = [1.5707963050, -0.2145988016, 0.0889789874, -0.0501743046,
          0.0308918810, -0.0170881256, 0.0066700901, -0.0012624911]


def cview(tile, start, step, count):
    """[P,C] tile -> [P,count] strided column view starting at col `start`."""
    c = tile[:, start:start + 1]
    a = [list(c.ap[0]), [step, count]]
    return bass.AP(c.tensor, c.offset, a)


def emit_core(nc, io):
    """Emit the whole per-core program. io: dict name->AP of DRAM tensors."""
    tab0, tab1 = io["tab0"], io["tab1"]      # [NK*4] f32 packed (x,y,d,0)
    ck_d = io["ck"]                          # [32] f32 consts
    sidx = io["sidx"]                        # [16,256] i32
    sir = io["sir"]                          # [256,5] i32
    cst_d = io["cst"]                        # [576] f32 host constants
    out_d = io["out"]                        # [2] f32

    with TileContext(nc) as tc, ExitStack() as ctx:
        pool = ctx.enter_context(tc.tile_pool(name="main", bufs=1))
        scr = ctx.enter_context(tc.tile_pool(name="scr", bufs=2))
        scrS = ctx.enter_context(tc.tile_pool(name="scrS", bufs=4))
        psum = ctx.enter_context(tc.tile_pool(name="psum", bufs=2, space="PSUM"))

        V = nc.vector
        G = nc.gpsimd
        A = nc.scalar

        def big(tag):
            return scr.tile([128, S], F32, name=tag, tag=tag)

        def sm(tag, c=1):
            return scrS.tile([128, c], F32, name=tag + str(c), tag=tag + str(c))

        # tensor_tensor_reduce with accum_out wedges the exec unit on this
        # hardware (sim passes); emit mult + tensor_reduce instead.
        def dot(acc, a, b, wid, tag):
            if wid > 32:
                tmp = scr.tile([128, wid], F32, name="dt" + tag, tag="dt" + tag)
            else:
                tmp = scrS.tile([128, wid], F32, name="dt" + tag, tag="dt" + tag)
            V.tensor_tensor(tmp[:, :], a, b, op=OP.mult)
            V.tensor_reduce(acc, tmp[:, :], axis=AX.X, op=OP.add)

        # ---- constants: one DMA row + PE ones-matmul broadcast (the old
        # dbl_bcast DMA chains serialized ~100us on the SP queue) ----
        ones1r = pool.tile([1, 128], F32)
        V.memset(ones1r[:, :], 1.0)
        ones1rh = pool.tile([1, 128], F16)
        V.memset(ones1rh[:, :], 1.0)

        cst_row = pool.tile([1, 576], F32)
        nc.sync.dma_start(out=cst_row[:, :], in_=cst_d[None, :])
        ck_row = pool.tile([1, 32], F32)
        nc.scalar.dma_start(out=ck_row[:, :], in_=ck_d[None, :])

        psC0 = psum.tile([128, 1024], F32, name="psC0", tag="psB")
        nc.tensor.matmul(psC0[:, 0:512], ones1r[:, :], cst_row[:, 0:512],
                         start=True, stop=True)
        nc.tensor.matmul(psC0[:, 512:576], ones1r[:, :], cst_row[:, 512:576],
                         start=True, stop=True)
        CST = pool.tile([128, 576], F32)
        V.tensor_copy(CST[:, 0:512], psC0[:, 0:512])
        V.tensor_copy(CST[:, 512:576], psC0[:, 512:576])
        psC1 = psum.tile([128, 1024], F32, name="psC1", tag="psB")
        nc.tensor.matmul(psC1[:, 0:32], ones1r[:, :], ck_row[:, :], start=True, stop=True)
        CK = pool.tile([128, 32], F32)
        V.tensor_copy(CK[:, :], psC1[:, 0:32])

        iotaS = CST[:, 0:256]    # broadcast row: value = col index
        GR = CST[:, 256:384]     # broadcast row: value = col // 16

        pcol = pool.tile([128, 1], F32)
        nc.sync.dma_start(out=pcol[:, :], in_=cst_d[256:384].rearrange("(n o) -> n o", o=1))
        BD = pool.tile([128, 128], F32)
        V.tensor_scalar(BD[:, :], GR[:, :], pcol[:, :], None, op0=OP.is_equal)
        pco = pool.tile([128, 1], F32)
        nc.scalar.dma_start(out=pco[:, :], in_=cst_d[0:128].rearrange("(n o) -> n o", o=1))
        SEL = pool.tile([32, 4], F32)   # SEL[p, d] = (p % 4 == d)
        nc.scalar.dma_start(out=SEL[:, :], in_=cst_d[384:512].rearrange("(p d) -> p d", d=4))
        pq4 = pool.tile([32, 1], F32)   # p // 4
        nc.sync.dma_start(out=pq4[:, :], in_=cst_d[512:544].rearrange("(n o) -> n o", o=1))

        # REP[t][m, p] = (p // 16 == m - 8t): replicates m-rows to hyp partitions
        pm16 = pool.tile([16, 1], F32)
        nc.scalar.dma_start(out=pm16[:, :], in_=cst_d[0:16].rearrange("(n o) -> n o", o=1))
        REP = {}
        for t in range(NTILE):
            pmt = pool.tile([16, 1], F32, name=f"pmt{t}", tag=f"pmt{t}")
            V.tensor_scalar(pmt[:, :], pm16[:, :], -8.0 * t, None, op0=OP.add)
            REP[t] = pool.tile([16, 128], F32, name=f"REP{t}", tag=f"REP{t}")
            V.tensor_scalar(REP[t][:, :], GR[0:16, :], pmt[:, :], None, op0=OP.is_equal)

        cbeta = pool.tile([128, 1], F32, name="cbeta")
        V.memset(cbeta[:, :], BETA)

        # ---- per-tile persistent state ----
        X = {}; Y = {}
        inl = {}; inl_fin = {}; pre = {}
        Rt = {}; Tt = {}; r2 = {}
        score = {}; lrk = {}; ltk = {}
        for t in range(NTILE):
            X[t] = [pool.tile([128, S], F32, tag=f"Xc{t}{i}", name=f"Xc{t}{i}") for i in range(3)]
            Y[t] = [pool.tile([128, S], F32, tag=f"Yc{t}{i}", name=f"Yc{t}{i}") for i in range(3)]
            inl[t] = pool.tile([128, S], F16, tag=f"inl_{t}", name=f"inl_{t}")
            inl_fin[t] = pool.tile([128, S], F16, tag=f"infin_{t}", name=f"infin_{t}")
            pre[t] = pool.tile([128, 1], F32, tag=f"pre_{t}", name=f"pre_{t}")
            Rt[t] = pool.tile([128, 9], F32, tag=f"Rt_{t}", name=f"Rt_{t}")
            Tt[t] = pool.tile([128, 3], F32, tag=f"Tt_{t}", name=f"Tt_{t}")
            r2[t] = pool.tile([128, S], F32, tag=f"r2_{t}", name=f"r2_{t}")
            score[t] = pool.tile([128, 1], F32, tag=f"score_{t}", name=f"score_{t}")
            lrk[t] = pool.tile([128, 1], F32, tag=f"lrk_{t}", name=f"lrk_{t}")
            ltk[t] = pool.tile([128, 1], F32, tag=f"ltk_{t}", name=f"ltk_{t}")

        # ---- decode + indirect-DMA gather (m-space) + backproject + replicate ----
        W32m = pool.tile([16, S], I32)
        nc.sync.dma_start(out=W32m[:, :], in_=sidx[:, :])
        i0m = pool.tile([16, S], I32)
        i1m = pool.tile([16, S], I32)
        V.tensor_scalar(i0m[:, :], W32m[:, :], 10, None, op0=OP.logical_shift_right)
        V.tensor_scalar(i1m[:, :], W32m[:, :], 1023, None, op0=OP.bitwise_and)

        if STAGE < 2:
            out2z = pool.tile([1, 2], F32)
            V.memset(out2z[:, :], 0.0)
            nc.sync.dma_start(out=out_d[None, :], in_=out2z[:, :])
            return
        # ---- two-level one-hot PE gather ----
        # idx = 128*c + r.  Per 512-col block: broadcast [r|c] rows to PSUM
        # via ones-matmul; ohr = (r==p) one-hot [128,512]; RG = tabs^T ohr
        # gives all 8 chunk candidates [(c,d), 512]; mask by (c==p//4) and
        # contract with SEL to pick the right chunk -> G [4, 512].
        # tabs[p, c, d]: table row (128c + p), channel d
        tabs0 = pool.tile([128, 32], F32)
        tabs1 = pool.tile([128, 32], F32)
        nc.sync.dma_start(out=tabs0[:, :].rearrange("p (c d) -> p c d", d=4),
                          in_=tab0.rearrange("(c p d) -> p c d", p=128, d=4))
        nc.scalar.dma_start(out=tabs1[:, :].rearrange("p (c d) -> p c d", d=4),
                          in_=tab1.rearrange("(c p d) -> p c d", p=128, d=4))

        # r/c rows in m-space, converted to f32
        rcm = {}
        for (tn, im) in ((0, i0m), (1, i1m)):
            rm = scr.tile([16, S], I32, name=f"rm{tn}", tag="rm")
            V.tensor_scalar(rm[:, :], im[:, :], 127, None, op0=OP.bitwise_and)
            cm = scr.tile([16, S], I32, name=f"cm{tn}", tag="cm")
            V.tensor_scalar(cm[:, :], im[:, :], 7, None, op0=OP.logical_shift_right)
            rf = scr.tile([16, S], F32, name=f"rf{tn}", tag="rf", bufs=2)
            V.tensor_copy(rf[:, :], rm[:, :])
            cf = scr.tile([16, S], F32, name=f"cf{tn}", tag="cf", bufs=2)
            V.tensor_copy(cf[:, :], cm[:, :])
            # r<128, c<8: exact in fp16 -> halves the staging bytes so the
            # flat tile double-buffers across tables in the same footprint
            rfh = scr.tile([16, S], F16, name=f"rfh{tn}", tag="rfh", bufs=2)
            V.tensor_copy(rfh[:, :], rf[:, :])
            cfh = scr.tile([16, S], F16, name=f"cfh{tn}", tag="cfh", bufs=2)
            V.tensor_copy(cfh[:, :], cf[:, :])
            rcm[tn] = (rfh, cfh)

        # Gm layout: [16, 4*S] channel-major (d * S + s)
        Gm0 = pool.tile([16, S * 4], F32)
        Gm1 = pool.tile([16, S * 4], F32)
        for (tn, tabs, Gm) in ((0, tabs0, Gm0), (1, tabs1, Gm1)):
            rf, cf = rcm[tn]
            # flat partition-0 staging: [r(4096) | c(4096)] (matmul moving
            # operands must live at base partition 0)
            rc = scr.tile([1, 8192], F16, name=f"rcfl{tn}", tag="rcfl", bufs=2)
            # split across two queues (SP + ACT) so the folds overlap
            nc.scalar.dma_start(out=rc[:, 0:4096], in_=rf[:, :])
            nc.sync.dma_start(out=rc[:, 4096:8192], in_=cf[:, :])
            Gs = scr.tile([4, 8 * 512], F32, name="Gs", tag="Gs", bufs=1)
            Gs = scr.tile([4, 8 * 512], F32, name="Gs", tag="Gs", bufs=1)
            for bb in range(8):          # 2 m-rows per block
                psb = psum.tile([128, 1024], F32, name="psb", tag="psB")
                nc.tensor.matmul(psb[:, 0:512], ones1rh[:, :],
                                 rc[:, 512 * bb:512 * (bb + 1)], start=True, stop=True)
                nc.tensor.matmul(psb[0:32, 512:1024], ones1rh[:, 0:32],
                                 rc[:, 4096 + 512 * bb:4096 + 512 * (bb + 1)],
                                 start=True, stop=True)
                ohr = scr.tile([128, 512], F32, name="ohr", tag="ohr")
                V.tensor_scalar(ohr[:, :], psb[:, 0:512], pco[:, :], None, op0=OP.is_equal)
                # c-mask does not depend on the RG matmul: compute it into
                # SBUF in parallel, then the mask-mult reads rg from PSUM
                # as its single PSUM input (no rg->SBUF copy needed)
                mc = scr.tile([32, 512], F32, name="mc", tag="mc")
                V.tensor_scalar(mc[:, :], psb[0:32, 512:1024], pq4[:, :], None,
                                op0=OP.is_equal)
                rg = psum.tile([32, 512], F32, name="rg", tag="psRG")
                nc.tensor.matmul(rg[:, :], tabs[:, :], ohr[:, :], start=True, stop=True)
                mskg = scr.tile([32, 512], F32, name="mskg", tag="mskg")
                V.tensor_tensor(mskg[:, :], rg[:, :], mc[:, :], op=OP.mult)
                g4 = psum.tile([4, 512], F32, name="g4", tag="psG4")
                nc.tensor.matmul(g4[:, :], SEL[:, :], mskg[:, :], start=True, stop=True)
                A.activation(Gs[:, 512 * bb:512 * (bb + 1)], g4[:, :], AF.Identity)
            # Gs [4, (16m, 256s)] -> Gm [16m, (4d, 256s)]: one DMA per channel
            for d in range(4):
                nc.sync.dma_start(out=Gm[:, d * S:(d + 1) * S], in_=Gs[d:d + 1, :])

        # backproject in m-space [16, S]
        Xm = [pool.tile([16, S], F32, name=f"Xm{i}", tag=f"Xm{i}") for i in range(3)]
        Ym = [pool.tile([16, S], F32, name=f"Ym{i}", tag=f"Ym{i}") for i in range(3)]
        CK16 = CK[0:16, :]
        for (Gt, dst, kc) in ((Gm0, Xm, 0), (Gm1, Ym, 9)):
            u = Gt[:, 0:S]; v = Gt[:, S:2 * S]; dd = Gt[:, 2 * S:3 * S]
            for i in range(3):
                a1 = scrS.tile([16, S], F32, name="mba", tag="mba")
                A.activation(a1[:, :], u, AF.Identity,
                             bias=CK16[:, kc + 3 * i + 2:kc + 3 * i + 3],
                             scale=CK16[:, kc + 3 * i:kc + 3 * i + 1])
                a2 = scrS.tile([16, S], F32, name="mbb", tag="mbb")
                V.scalar_tensor_tensor(a2[:, :], v, CK16[:, kc + 3 * i + 1:kc + 3 * i + 2],
                                       a1[:, :], op0=OP.mult, op1=OP.add)
                V.tensor_tensor(dst[i][:, :], a2[:, :], dd, op=OP.mult)
        nX2m = pool.tile([16, S], F32)
        nY2m = pool.tile([16, S], F32)
        for (src3, dstn) in ((Xm, nX2m), (Ym, nY2m)):
            s1 = scrS.tile([16, S], F32, name="mna", tag="mna")
            s2 = scrS.tile([16, S], F32, name="mnb", tag="mnb")
            s3 = scrS.tile([16, S], F32, name="mnc", tag="mnc")
            A.activation(s1[:, :], src3[0][:, :], AF.Square)
            A.activation(s2[:, :], src3[1][:, :], AF.Square)
            A.activation(s3[:, :], src3[2][:, :], AF.Square)
            V.tensor_tensor(s1[:, :], s1[:, :], s2[:, :], op=OP.add)
            V.tensor_tensor(dstn[:, :], s1[:, :], s3[:, :], op=OP.add)

        # ---- transposed per-m quantity tables for PE fit-reductions ----
        # SPt[k][s_local, m*18+q]: for s-half k, per-m column group of the 18
        # per-hypothesis reduction integrands evaluated at sample s:
        # q=0: 1, q=1..3: X_i, q=4..6: Y_i, q=7..15: X_i*Y_j, 16: |X|^2, 17: |Y|^2
        I16 = pool.tile([16, 16], F32)
        V.tensor_scalar(I16[:, :], iotaS[0:16, 0:16], pm16[:, :], None, op0=OP.is_equal)
        I128 = pool.tile([128, 128], F16)
        V.tensor_scalar(I128[:, :], CST[:, 0:128], pco[:, :], None, op0=OP.is_equal)
        mk8 = pool.tile([128, 8], F32)   # mk8[p, mm] = (p//16 == mm)
        V.tensor_scalar(mk8[:, :], iotaS[:, 0:8], pcol[:, :], None, op0=OP.is_equal)

        SPt = [pool.tile([128, 16 * 18], F32, name=f"SPt{k}", tag=f"SPt{k}")
               for k in range(2)]
        msrc = [Xm[0], Xm[1], Xm[2], Ym[0], Ym[1], Ym[2], nX2m, nY2m]
        mq = [1, 2, 3, 4, 5, 6, 16, 17]
        for k in range(2):
            V.memset(cview(SPt[k], 0, 18, 16), 1.0)
            for (srcq, q) in zip(msrc, mq):
                pst = psum.tile([128, 16], F32, name="psTq", tag="psG4")
                nc.tensor.matmul(pst[:, :], srcq[:, 128 * k:128 * (k + 1)], I16[:, :],
                                 start=True, stop=True)
                A.activation(cview(SPt[k], q, 18, 16), pst[:, :], AF.Identity)
            for i in range(3):
                for j in range(3):
                    V.tensor_tensor(cview(SPt[k], 7 + 3 * i + j, 18, 16),
                                    cview(SPt[k], 1 + i, 18, 16),
                                    cview(SPt[k], 4 + j, 18, 16), op=OP.mult)

        # replicate m-space -> hypothesis partitions via PE one-hot matmul
        for t in range(NTILE):
            reps = ([(Xm[i], X[t][i]) for i in range(3)] +
                    [(Ym[i], Y[t][i]) for i in range(3)])
            for (srcq, dstq) in reps:
                PS = psum.tile([128, S], F32, name="PSrep", tag="psB")
                nc.tensor.matmul(PS[:, :], REP[t][:, :], srcq[:, :], start=True, stop=True)
                A.activation(dstq[:, :], PS[:, :], AF.Identity)

            sirT = scr.tile([128, NCON], I32, tag="sirT", name="sirT")
            nc.sync.dma_start(out=sirT[:, :], in_=sir[128 * t:128 * (t + 1), :])
            sirF = scr.tile([128, NCON], F32, tag="sirF", name="sirF")
            V.tensor_copy(sirF[:, :], sirT[:, :])
            V.tensor_scalar(inl[t][:, :], iotaS[:, :], sirF[:, 0:1], None, op0=OP.is_equal)
            for k in range(1, NCON):
                c = scr.tile([128, S], F16, name="icmp", tag="icmp")
                V.tensor_scalar(c[:, :], iotaS[:, :], sirF[:, k:k + 1], None, op0=OP.is_equal)
                V.tensor_tensor(inl[t][:, :], inl[t][:, :], c[:, :], op=OP.max)
            V.tensor_copy(inl_fin[t][:, :], inl[t][:, :])
            V.memset(pre[t][:, :], float(NCON))


        if STAGE < 4:
            out2z = pool.tile([1, 2], F32)
            V.memset(out2z[:, :], 0.0)
            nc.sync.dma_start(out=out_d[None, :], in_=out2z[:, :])
            return
        # ---- weighted procrustes via quaternion-Newton ----
        def fit(wmap):
            for t in range(NTILE):
                w = wmap[t]
                # PE-side weighted reductions: transpose w, then one matmul
                # against the per-m integrand table; per-hypothesis rows are
                # extracted with the p//16 == mm masks.
                wT = []
                for k in range(2):
                    pstw = psum.tile([128, 128], F32, name="pstw", tag="psRG")
                    nc.tensor.matmul(pstw[:, :], w[:, 128 * k:128 * (k + 1)],
                                     I128[:, :], start=True, stop=True)
                    wTk = scr.tile([128, 128], F32, name=f"wT{k}", tag=f"wT{k}", bufs=2)
                    A.activation(wTk[:, :], pstw[:, :], AF.Identity)
                    wT.append(wTk)
                psF = psum.tile([128, 144], F32, name="psF", tag="psB")
                for k in range(2):
                    nc.tensor.matmul(psF[:, :], wT[k][:, :],
                                     SPt[k][:, 144 * t:144 * (t + 1)],
                                     start=(k == 0), stop=(k == 1))
                FH = scrS.tile([128, 18], F32, name="FH", tag="FH18", bufs=2)
                V.tensor_scalar(FH[:, :], psF[:, 0:18], mk8[:, 0:1], None, op0=OP.mult)
                for mm in range(1, 8):
                    V.scalar_tensor_tensor(FH[:, :], psF[:, 18 * mm:18 * (mm + 1)],
                                           mk8[:, mm:mm + 1], FH[:, :],
                                           op0=OP.mult, op1=OP.add)
                sw = sm("sw")
                V.tensor_scalar(sw[:, :], FH[:, 0:1], 1e-8, None, op0=OP.add)
                inv = sm("inv"); V.reciprocal(inv[:, :], sw[:, :])
                swX = FH[:, 1:4]; swY = FH[:, 4:7]
                Hp = FH[:, 7:16]
                GA = sm("GA"); GB = sm("GB")
                V.tensor_copy(GA[:, :], FH[:, 16:17])
                V.tensor_copy(GB[:, :], FH[:, 17:18])

                cx = sm("cx", 3); cy = sm("cy", 3)
                V.tensor_scalar(cx[:, :], swX[:, :], inv[:, :], None, op0=OP.mult)
                V.tensor_scalar(cy[:, :], swY[:, :], inv[:, :], None, op0=OP.mult)
                # H = Hp - sw * cx (x) cy
                E = sm("E", 9)
                for i_ in range(3):
                    V.tensor_scalar(E[:, 3 * i_:3 * i_ + 3], cy[:, :], cx[:, i_:i_ + 1],
                                    None, op0=OP.mult)
                V.tensor_scalar(E[:, :], E[:, :], sw[:, :], None, op0=OP.mult)
                H = sm("H", 9)
                V.tensor_tensor(H[:, :], Hp[:, :], E[:, :], op=OP.subtract)
                # GA/GB centered, clamped; lam0 = sqrt(GA*GB)
                g1 = sm("g1")
                dot(g1[:, :], cx[:, :], swX[:, :], 3, "g1")
                V.tensor_tensor(GA[:, :], GA[:, :], g1[:, :], op=OP.subtract)
                V.tensor_scalar(GA[:, :], GA[:, :], 0.0, None, op0=OP.max)
                g2 = sm("g2")
                dot(g2[:, :], cy[:, :], swY[:, :], 3, "g2")
                V.tensor_tensor(GB[:, :], GB[:, :], g2[:, :], op=OP.subtract)
                V.tensor_scalar(GB[:, :], GB[:, :], 0.0, None, op0=OP.max)
                lam = sm("lam0")
                V.tensor_tensor(lam[:, :], GA[:, :], GB[:, :], op=OP.add)
                V.tensor_scalar(lam[:, :], lam[:, :], 0.5, None, op0=OP.mult)

                # quartic coefficients
                c2s = sm("c2s")
                dot(c2s[:, :], H[:, :], H[:, :], 9, "c2s")
                C2 = sm("C2"); C2x2 = sm("C2x2")
                V.tensor_scalar(C2[:, :], c2s[:, :], -2.0, None, op0=OP.mult)
                V.tensor_scalar(C2x2[:, :], c2s[:, :], -4.0, None, op0=OP.mult)
                # detH -> C1
                a0 = sm("a0"); a1_ = sm("a1"); a2_ = sm("a2")
                mt = sm("mt")
                V.tensor_tensor(mt[:, :], H[:, 4:5], H[:, 8:9], op=OP.mult)
                V.scalar_tensor_tensor(a0[:, :], H[:, 5:6], H[:, 7:8], mt[:, :],
                                       op0=OP.mult, op1=OP.subtract)
                V.tensor_scalar(a0[:, :], a0[:, :], -1.0, None, op0=OP.mult)
                V.tensor_tensor(mt[:, :], H[:, 3:4], H[:, 8:9], op=OP.mult)
                V.scalar_tensor_tensor(a1_[:, :], H[:, 5:6], H[:, 6:7], mt[:, :],
                                       op0=OP.mult, op1=OP.subtract)
                V.tensor_scalar(a1_[:, :], a1_[:, :], -1.0, None, op0=OP.mult)
                V.tensor_tensor(mt[:, :], H[:, 3:4], H[:, 7:8], op=OP.mult)
                V.scalar_tensor_tensor(a2_[:, :], H[:, 4:5], H[:, 6:7], mt[:, :],
                                       op0=OP.mult, op1=OP.subtract)
                V.tensor_scalar(a2_[:, :], a2_[:, :], -1.0, None, op0=OP.mult)
                d0 = sm("d0")
                V.tensor_tensor(d0[:, :], H[:, 0:1], a0[:, :], op=OP.mult)
                e1 = sm("e1")
                V.scalar_tensor_tensor(e1[:, :], a1_[:, :], H[:, 1:2], d0[:, :],
                                       op0=OP.mult, op1=OP.subtract)  # H1*a1 - H0*a0
                e2 = sm("e2")
                V.scalar_tensor_tensor(e2[:, :], a2_[:, :], H[:, 2:3], e1[:, :],
                                       op0=OP.mult, op1=OP.subtract)  # detH
                C1 = sm("C1")
                V.tensor_scalar(C1[:, :], e2[:, :], -8.0, None, op0=OP.mult)

                # N matrix [128,16] row-major (upper triangle only)
                Nt = sm("Nt", 16)
                V.memset(Nt[:, :], 0.0)
                V.tensor_tensor(Nt[:, 0:1], H[:, 0:1], H[:, 4:5], op=OP.add)
                V.tensor_tensor(Nt[:, 0:1], Nt[:, 0:1], H[:, 8:9], op=OP.add)
                V.scalar_tensor_tensor(Nt[:, 5:6], H[:, 0:1], 2.0, Nt[:, 0:1],
                                       op0=OP.mult, op1=OP.subtract)
                V.scalar_tensor_tensor(Nt[:, 10:11], H[:, 4:5], 2.0, Nt[:, 0:1],
                                       op0=OP.mult, op1=OP.subtract)
                V.scalar_tensor_tensor(Nt[:, 15:16], H[:, 8:9], 2.0, Nt[:, 0:1],
                                       op0=OP.mult, op1=OP.subtract)

                # N is symmetric: store the upper triangle only and
                # canonicalize all reads (saves 6 mirror copies)
                def NTC(r_, c_):
                    a_, b_ = (r_, c_) if r_ <= c_ else (c_, r_)
                    return Nt[:, 4 * a_ + b_:4 * a_ + b_ + 1]

                def KTC(r_, c_):
                    a_, b_ = (r_, c_) if r_ <= c_ else (c_, r_)
                    return Kt[:, 4 * a_ + b_:4 * a_ + b_ + 1]

                def offd(i, j, ca, cb, op):
                    V.tensor_tensor(Nt[:, 4 * i + j:4 * i + j + 1], H[:, ca:ca + 1],
                                    H[:, cb:cb + 1], op=op)
                offd(0, 1, 5, 7, OP.subtract)   # Syz-Szy
                offd(0, 2, 6, 2, OP.subtract)   # Szx-Sxz
                offd(0, 3, 1, 3, OP.subtract)   # Sxy-Syx
                offd(1, 2, 1, 3, OP.add)        # Sxy+Syx
                offd(1, 3, 6, 2, OP.add)        # Szx+Sxz
                offd(2, 3, 5, 7, OP.add)        # Syz+Szy

                # C0 = det(N): Laplace rows (0,1) x (2,3)
                prs = [(0, 1), (0, 2), (0, 3), (1, 2), (1, 3), (2, 3)]
                Mtop = sm("Mtop", 6); Mbot = sm("Mbot", 6)
                for kk, (a_, b_) in enumerate(prs):
                    p = sm("lp")
                    V.tensor_tensor(p[:, :], NTC(0, a_), NTC(1, b_), op=OP.mult)
                    q_ = sm("lq")
                    V.scalar_tensor_tensor(q_[:, :], NTC(1, a_),
                                           NTC(0, b_), p[:, :], op0=OP.mult, op1=OP.subtract)
                    V.tensor_scalar(Mtop[:, kk:kk + 1], q_[:, :], -1.0, None, op0=OP.mult)
                    p2 = sm("lp2")
                    V.tensor_tensor(p2[:, :], NTC(2, a_), NTC(3, b_), op=OP.mult)
                    q2_ = sm("lq2")
                    V.scalar_tensor_tensor(q2_[:, :], NTC(3, a_),
                                           NTC(2, b_), p2[:, :], op0=OP.mult, op1=OP.subtract)
                    V.tensor_scalar(Mbot[:, kk:kk + 1], q2_[:, :], -1.0, None, op0=OP.mult)
                cc = sm("cc", 6)
                # det = M01*m23 - M02*m13 + M03*m12 + M12*m03 - M13*m02 + M23*m01
                V.tensor_tensor(cc[:, 0:1], Mtop[:, 0:1], Mbot[:, 5:6], op=OP.mult)
                V.tensor_tensor(cc[:, 1:2], Mtop[:, 1:2], Mbot[:, 4:5], op=OP.mult)
                V.tensor_tensor(cc[:, 2:3], Mtop[:, 2:3], Mbot[:, 3:4], op=OP.mult)
                V.tensor_tensor(cc[:, 3:4], Mtop[:, 3:4], Mbot[:, 2:3], op=OP.mult)
                V.tensor_tensor(cc[:, 4:5], Mtop[:, 4:5], Mbot[:, 1:2], op=OP.mult)
                V.tensor_tensor(cc[:, 5:6], Mtop[:, 5:6], Mbot[:, 0:1], op=OP.mult)
                C0 = sm("C0"); s1_ = sm("cs1"); s2_ = sm("cs2")
                V.tensor_tensor(s1_[:, :], cc[:, 0:1], cc[:, 1:2], op=OP.subtract)
                V.tensor_tensor(s2_[:, :], cc[:, 2:3], cc[:, 3:4], op=OP.add)
                V.tensor_tensor(s1_[:, :], s1_[:, :], s2_[:, :], op=OP.add)
                V.tensor_tensor(s1_[:, :], s1_[:, :], cc[:, 4:5], op=OP.subtract)
                V.tensor_tensor(C0[:, :], s1_[:, :], cc[:, 5:6], op=OP.add)

                # Newton on P(l) = l^4 + C2 l^2 + C1 l + C0
                for _ in range(NEWTON):
                    e = sm("ne"); Av = sm("nA"); Bv = sm("nB"); D = sm("nD"); P = sm("nP")
                    Ev = sm("nE"); Fv = sm("nF"); Pp = sm("nPp"); gq = sm("ng")
                    V.tensor_tensor(e[:, :], lam[:, :], lam[:, :], op=OP.mult)
                    V.tensor_tensor(Av[:, :], e[:, :], C2[:, :], op=OP.add)
                    V.tensor_tensor(Bv[:, :], Av[:, :], e[:, :], op=OP.mult)
                    V.scalar_tensor_tensor(D[:, :], lam[:, :], C1[:, :], C0[:, :],
                                           op0=OP.mult, op1=OP.add)
                    V.tensor_tensor(P[:, :], Bv[:, :], D[:, :], op=OP.add)
                    V.scalar_tensor_tensor(Ev[:, :], e[:, :], 4.0, C2x2[:, :],
                                           op0=OP.mult, op1=OP.add)
                    V.tensor_tensor(Fv[:, :], Ev[:, :], lam[:, :], op=OP.mult)
                    V.tensor_tensor(Pp[:, :], Fv[:, :], C1[:, :], op=OP.add)
                    V.tensor_scalar(Pp[:, :], Pp[:, :], 1e-30, None, op0=OP.max)
                    V.reciprocal(Pp[:, :], Pp[:, :])
                    V.tensor_tensor(gq[:, :], P[:, :], Pp[:, :], op=OP.mult)
                    lam2 = sm("nlam")
                    V.tensor_tensor(lam2[:, :], lam[:, :], gq[:, :], op=OP.subtract)
                    lam = lam2

                # K = N - lam I
                Kt = sm("Kt", 16)
                V.tensor_copy(Kt[:, :], Nt[:, :])
                for d_ in range(4):
                    V.tensor_tensor(Kt[:, 5 * d_:5 * d_ + 1], Nt[:, 5 * d_:5 * d_ + 1],
                                    lam[:, :], op=OP.subtract)

                # adjugate-row candidates: the 4 dets per candidate share the
                # 2x2 minors of one row pair -- precompute all 6 per pair
                prs2 = [(0, 1), (0, 2), (0, 3), (1, 2), (1, 3), (2, 3)]

                def minors6(r1, rr2, tag):
                    # MM[:, k] = K[rr2,u]*K[r1,v] - K[r1,u]*K[rr2,v] = -minor(u,v)
                    MM = sm("MM" + tag, 6)
                    for k2, (u, vv) in enumerate(prs2):
                        p_ = sm("mmp" + tag)
                        V.tensor_tensor(p_[:, :], KTC(r1, u),
                                        KTC(rr2, vv), op=OP.mult)
                        V.scalar_tensor_tensor(MM[:, k2:k2 + 1],
                                               KTC(rr2, u),
                                               KTC(r1, vv), p_[:, :],
                                               op0=OP.mult, op1=OP.subtract)
                    return MM

                def det3row(out_col, r0, MM, cols, sgn):
                    (ca, cb, cc_) = cols
                    mbc = MM[:, prs2.index((cb, cc_)):prs2.index((cb, cc_)) + 1]
                    mac = MM[:, prs2.index((ca, cc_)):prs2.index((ca, cc_)) + 1]
                    mab = MM[:, prs2.index((ca, cb)):prs2.index((ca, cb)) + 1]
                    # det = -(K[r0,ca]*mbc) + K[r0,cb]*mac - K[r0,cc]*mab
                    z1 = sm("z1")
                    V.tensor_tensor(z1[:, :], KTC(r0, ca), mbc,
                                    op=OP.mult)
                    zb = sm("zb")
                    V.tensor_tensor(zb[:, :], KTC(r0, cb), mac,
                                    op=OP.mult)
                    zr = sm("zr")
                    V.tensor_tensor(zr[:, :], zb[:, :], z1[:, :], op=OP.subtract)
                    z3 = sm("z3")
                    V.tensor_tensor(z3[:, :], KTC(r0, cc_), mab,
                                    op=OP.mult)
                    if sgn < 0:
                        V.scalar_tensor_tensor(out_col, zr[:, :], -1.0, z3[:, :],
                                               op0=OP.mult, op1=OP.add)
                    else:
                        V.tensor_tensor(out_col, zr[:, :], z3[:, :], op=OP.subtract)

                qa = sm("qa", 4); qb = sm("qb", 4)
                MMa = minors6(1, 2, "a")
                MMb = minors6(2, 3, "b")
                allc = [0, 1, 2, 3]
                for i in range(4):
                    cols = tuple(cq for cq in allc if cq != i)
                    det3row(qa[:, i:i + 1], 0, MMa, cols, +1 if (3 + i) % 2 == 0 else -1)
                    det3row(qb[:, i:i + 1], 1, MMb, cols, +1 if i % 2 == 0 else -1)

                na = sm("na"); nb = sm("nb")
                dot(na[:, :], qa[:, :], qa[:, :], 4, "na")
                dot(nb[:, :], qb[:, :], qb[:, :], 4, "nb")
                msk = sm("msk")
                V.tensor_tensor(msk[:, :], na[:, :], nb[:, :], op=OP.is_ge)
                qd = sm("qd", 4); q = sm("q", 4)
                V.tensor_tensor(qd[:, :], qa[:, :], qb[:, :], op=OP.subtract)
                V.scalar_tensor_tensor(q[:, :], qd[:, :], msk[:, :], qb[:, :],
                                       op0=OP.mult, op1=OP.add)
                # R from UNNORMALIZED quaternion: fold 1/|q|^2 into the
                # quadratic products (no sqrt on ACT; degenerate q -> i2=0
                # gives R = I for free)
                n2 = sm("n2")
                dot(n2[:, :], q[:, :], q[:, :], 4, "n2")
                i2 = sm("i2")
                V.tensor_scalar(i2[:, :], n2[:, :], 1e-30, None, op0=OP.add)
                V.reciprocal(i2[:, :], i2[:, :])
                ndg = sm("ndg")
                V.tensor_scalar(ndg[:, :], n2[:, :], 1e-24, None, op0=OP.is_ge)
                V.tensor_tensor(i2[:, :], i2[:, :], ndg[:, :], op=OP.mult)
                i2m2 = sm("i2m2"); i2p2 = sm("i2p2")
                V.tensor_scalar(i2m2[:, :], i2[:, :], -2.0, None, op0=OP.mult)
                V.tensor_scalar(i2p2[:, :], i2[:, :], 2.0, None, op0=OP.mult)

                gg1 = sm("gg1", 3); gg2 = sm("gg2", 2); xz = sm("xz"); gg3 = sm("gg3", 3)
                V.tensor_tensor(gg1[:, :], q[:, 1:4], q[:, 1:4], op=OP.mult)     # xx,yy,zz
                V.tensor_tensor(gg2[:, :], q[:, 1:3], q[:, 2:4], op=OP.mult)     # xy,yz
                V.tensor_tensor(xz[:, :], q[:, 1:2], q[:, 3:4], op=OP.mult)
                V.tensor_scalar(gg3[:, :], q[:, 1:4], q[:, 0:1], None, op0=OP.mult)  # wx,wy,wz
                Rl = Rt[t]
                sd = sm("sd")
                V.tensor_tensor(sd[:, :], gg1[:, 1:2], gg1[:, 2:3], op=OP.add)
                V.tensor_scalar(Rl[:, 0:1], sd[:, :], i2m2[:, :], 1.0, op0=OP.mult, op1=OP.add)
                sd2 = sm("sd2")
                V.tensor_tensor(sd2[:, :], gg1[:, 0:1], gg1[:, 2:3], op=OP.add)
                V.tensor_scalar(Rl[:, 4:5], sd2[:, :], i2m2[:, :], 1.0, op0=OP.mult, op1=OP.add)
                sd3 = sm("sd3")
                V.tensor_tensor(sd3[:, :], gg1[:, 0:1], gg1[:, 1:2], op=OP.add)
                V.tensor_scalar(Rl[:, 8:9], sd3[:, :], i2m2[:, :], 1.0, op0=OP.mult, op1=OP.add)

                def offR(col, pa, pb, op, tag):
                    u_ = sm("oR" + tag)
                    V.tensor_tensor(u_[:, :], pa, pb, op=op)
                    V.tensor_scalar(Rl[:, col:col + 1], u_[:, :], i2p2[:, :], None, op0=OP.mult)
                offR(1, gg2[:, 0:1], gg3[:, 2:3], OP.subtract, "a")  # xy-wz
                offR(3, gg2[:, 0:1], gg3[:, 2:3], OP.add, "b")       # xy+wz
                offR(2, xz[:, :], gg3[:, 1:2], OP.add, "c")          # xz+wy
                offR(6, xz[:, :], gg3[:, 1:2], OP.subtract, "d")     # xz-wy
                offR(5, gg2[:, 1:2], gg3[:, 0:1], OP.subtract, "e")  # yz-wx
                offR(7, gg2[:, 1:2], gg3[:, 0:1], OP.add, "f")       # yz+wx

                for i in range(3):
                    dm = sm(f"tdm{i}")
                    dot(dm[:, :], Rl[:, 3 * i:3 * i + 3], cx[:, :], 3, f"tdm{i}")
                    V.tensor_tensor(Tt[t][:, i:i + 1], cy[:, i:i + 1], dm[:, :], op=OP.subtract)

        def resid2(on_act=True):
            # on_act: ACT (idle during fits) takes the R-row init and the
            # squares (Identity/Square share one act-func set). The final
            # call runs DVE-only: ACT is the binder in the scoring tail.
            for t in range(NTILE):
                Rl = Rt[t]; Tl = Tt[t]
                sqs = []
                for i in range(3):
                    a1 = big("ra")
                    if on_act:
                        A.activation(a1[:, :], X[t][0][:, :], AF.Identity,
                                     bias=Tl[:, i:i + 1], scale=Rl[:, 3 * i:3 * i + 1])
                    else:
                        V.tensor_scalar(a1[:, :], X[t][0][:, :], Rl[:, 3 * i:3 * i + 1],
                                        Tl[:, i:i + 1], op0=OP.mult, op1=OP.add)
                    V.scalar_tensor_tensor(a1[:, :], X[t][1][:, :], Rl[:, 3 * i + 1:3 * i + 2],
                                           a1[:, :], op0=OP.mult, op1=OP.add)
                    V.scalar_tensor_tensor(a1[:, :], X[t][2][:, :], Rl[:, 3 * i + 2:3 * i + 3],
                                           a1[:, :], op0=OP.mult, op1=OP.add)
                    di = big(f"rd{i}")
                    V.tensor_tensor(di[:, :], Y[t][i][:, :], a1[:, :], op=OP.subtract)
                    sq = big(f"rq{i}")
                    if on_act:
                        A.activation(sq[:, :], di[:, :], AF.Square)
                    else:
                        V.tensor_tensor(sq[:, :], di[:, :], di[:, :], op=OP.mult)
                    sqs.append(sq)
                V.tensor_tensor(r2[t][:, :], sqs[0][:, :], sqs[1][:, :], op=OP.add)
                V.tensor_tensor(r2[t][:, :], r2[t][:, :], sqs[2][:, :], op=OP.add)

        if STAGE < 5:
            fit(inl)
            out2z = pool.tile([1, 2], F32)
            V.memset(out2z[:, :], 0.0)
            nc.sync.dma_start(out=out_d[None, :], in_=out2z[:, :])
            return
        # ---- refinement loop ----
        for it in range(NREF):
            fit(inl)
            resid2()
            for t in range(NTILE):
                refm = scr.tile([128, S], F16, name="refm", tag="refm")
                V.tensor_scalar(refm[:, :], r2[t][:, :], TH * TH, None, op0=OP.is_lt)
                rsum = sm("rsum")
                V.tensor_reduce(rsum[:, :], refm[:, :], axis=AX.X, op=OP.add)
                imp = sm("impf")
                V.tensor_tensor(imp[:, :], rsum[:, :], pre[t][:, :], op=OP.is_gt)
                dpre = sm("dpre")
                V.tensor_tensor(dpre[:, :], rsum[:, :], pre[t][:, :], op=OP.subtract)
                if it < NREF - 1:
                    pre2 = scrS.tile([128, 1], F32, tag="pre2", name="pre2")
                    V.scalar_tensor_tensor(pre2[:, :], dpre[:, :], imp[:, :], pre[t][:, :],
                                           op0=OP.mult, op1=OP.add)
                    pre[t] = pre2
                dbig = scr.tile([128, S], F16, name="dblend", tag="dblend")
                V.tensor_tensor(dbig[:, :], inl[t][:, :], inl_fin[t][:, :], op=OP.subtract)
                nf = scr.tile([128, S], F16, tag="nfin", name="nfin", bufs=4)
                V.tensor_scalar(dbig[:, :], dbig[:, :], imp[:, :], None, op0=OP.mult)
                V.tensor_tensor(nf[:, :], dbig[:, :], inl_fin[t][:, :], op=OP.add)
                inl_fin[t] = nf
                if it < NREF - 1:
                    dbig2 = scr.tile([128, S], F16, name="dblend2", tag="dblend2")
                    V.tensor_tensor(dbig2[:, :], refm[:, :], inl[t][:, :], op=OP.subtract)
                    ni = scr.tile([128, S], F16, tag="ninl", name="ninl", bufs=4)
                    V.tensor_scalar(dbig2[:, :], dbig2[:, :], imp[:, :], None, op0=OP.mult)
                    V.tensor_tensor(ni[:, :], dbig2[:, :], inl[t][:, :], op=OP.add)
                    inl[t] = ni

        if STAGE < 6:
            out2z = pool.tile([1, 2], F32)
            V.memset(out2z[:, :], 0.0)
            nc.sync.dma_start(out=out_d[None, :], in_=out2z[:, :])
            return
        # ---- final fit + scoring + losses ----
        fit(inl_fin)
        resid2(on_act=False)
        out2 = pool.tile([1, 2], F32)
        # sweep 1: everything needing the Sqrt act-table (+ DVE), both tiles
        r_ = {}; rotp = {}; te_ = {}
        for t in range(NTILE):
            r_[t] = scr.tile([128, S], F32, tag="rfin", name="rfin", bufs=2)
            A.activation(r_[t][:, :], r2[t][:, :], AF.Sqrt)

            tr = sm("tr")
            dot(tr[:, :], Rt[t][:, :], CK[:, 18:27], 9, "tr")
            c = sm("cl")
            V.tensor_scalar(c[:, :], tr[:, :], 0.5, -0.5, op0=OP.mult, op1=OP.add)
            V.tensor_scalar(c[:, :], c[:, :], -1.0 + 1e-6, None, op0=OP.max)
            V.tensor_scalar(c[:, :], c[:, :], 1.0 - 1e-6, None, op0=OP.min)
            aab = sm("aab")
            V.tensor_scalar(aab[:, :].bitcast(mybir.dt.uint32), c[:, :].bitcast(mybir.dt.uint32),
                            0x7FFFFFFF, None, op0=OP.bitwise_and)
            p = sm("acp")
            V.tensor_scalar(p[:, :], aab[:, :], ACOS_C[7], ACOS_C[6], op0=OP.mult, op1=OP.add)
            for cf in ACOS_C[5::-1]:
                V.tensor_scalar(p[:, :], p[:, :], aab[:, :], cf, op0=OP.mult, op1=OP.add)
            om = sm("om")
            V.tensor_scalar(om[:, :], aab[:, :], -1.0, 1.0, op0=OP.mult, op1=OP.add)
            A.activation(om[:, :], om[:, :], AF.Sqrt)
            apos = sm("apos")
            V.tensor_tensor(apos[:, :], p[:, :], om[:, :], op=OP.mult)
            mskn = sm("mskn")
            V.tensor_scalar(mskn[:, :], c[:, :], 0.0, None, op0=OP.is_lt)
            uu = sm("uu")
            V.tensor_scalar(uu[:, :], apos[:, :], -2.0, PI, op0=OP.mult, op1=OP.add)
            vv = sm("vvl")
            V.tensor_tensor(vv[:, :], mskn[:, :], uu[:, :], op=OP.mult)
            ac = sm("acos")
            V.tensor_tensor(ac[:, :], apos[:, :], vv[:, :], op=OP.add)
            rotp[t] = sm("rot")
            V.tensor_scalar(rotp[t][:, :], ac[:, :], (180.0 / PI) / MAX_ROT, None, op0=OP.mult)

            dt3 = sm("dt3", 3)
            V.tensor_tensor(dt3[:, :], Tt[t][:, :], CK[:, 27:30], op=OP.subtract)
            te2 = sm("te2")
            dot(te2[:, :], dt3[:, :], dt3[:, :], 3, "te2")
            te_[t] = sm("te")
            A.activation(te_[t][:, :], te2[:, :], AF.Sqrt)
        # sweep 2: Sigmoid-only act-table, both tiles (tanh(x) = 2*sig(2x)-1
        # exactly -- Tanh would pull a different act-func set than Sigmoid)
        for t in range(NTILE):
            sg = big("sgm")
            A.activation(sg[:, :], r_[t][:, :], AF.Sigmoid, bias=cbeta[:, :], scale=-BETA / TH)
            V.tensor_reduce(score[t][:, :], sg[:, :], axis=AX.X, op=OP.add)
            rot = sm("rotT")
            A.activation(rot[:, :], rotp[t][:, :], AF.Sigmoid, scale=2.0)
            V.tensor_scalar(lrk[t][:, :], rot[:, :], 2.0 * MAX_ROT, -MAX_ROT,
                            op0=OP.mult, op1=OP.add)
            A.activation(ltk[t][:, :], te_[t][:, :], AF.Sigmoid, scale=2.0 / MAX_TRANS)
            V.tensor_scalar(ltk[t][:, :], ltk[t][:, :], 2.0 * MAX_TRANS, -MAX_TRANS,
                            op0=OP.mult, op1=OP.add)

        # softmax over 16-hypothesis groups + total reduction (PE ones-matmul)
        ones1 = pool.tile([128, 1], F32)
        V.memset(ones1[:, :], 1.0)
        tot = {}
        for t in range(NTILE):
            eS = sm("eS")
            A.activation(eS[:, :], score[t][:, :], AF.Exp, scale=1.0 / TEMP)
            ps = psum.tile([128, 1], F32, name="psG", tag="psRG")
            nc.tensor.matmul(ps[:, :], BD[:, :], eS[:, :], start=True, stop=True)
            wgt = sm("wgt")
            V.reciprocal(wgt[:, :], ps[:, :])
            V.tensor_tensor(wgt[:, :], eS[:, :], wgt[:, :], op=OP.mult)
            lw2 = sm("lw2", 2)
            V.tensor_tensor(lw2[:, 0:1], lrk[t][:, :], wgt[:, :], op=OP.mult)
            V.tensor_tensor(lw2[:, 1:2], ltk[t][:, :], wgt[:, :], op=OP.mult)
            ps2 = psum.tile([1, 2], F32, name="psT", tag="psG4")
            nc.tensor.matmul(ps2[:, :], ones1[:, :], lw2[:, :], start=True, stop=True)
            tt2 = pool.tile([1, 2], F32, name=f"tt2_{t}", tag=f"tt2_{t}")
            V.tensor_copy(tt2[:, :], ps2[:, :])
            tot[t] = tt2
        sr = pool.tile([1, 1], F32)
        st = pool.tile([1, 1], F32)
        V.tensor_tensor(sr[:, :], tot[0][:, 0:1], tot[1][:, 0:1], op=OP.add)
        V.tensor_tensor(st[:, :], tot[0][:, 1:2], tot[1][:, 1:2], op=OP.add)
        V.tensor_scalar(out2[:, 0:1], sr[:, :], 1.0 / IT_M, None, op0=OP.mult)
        V.tensor_scalar(out2[:, 1:2], st[:, :], 1.0 / IT_M, None, op0=OP.mult)
        nc.sync.dma_start(out=out_d[None, :], in_=out2[:, :])


def build_program():
    nc = bacc.Bacc("TRN2", target_bir_lowering=False, debug=False, num_devices=B)
    io = {
        "tab0": nc.dram_tensor("tab0", [NK * 4], F32, kind="ExternalInput").ap(),
        "tab1": nc.dram_tensor("tab1", [NK * 4], F32, kind="ExternalInput").ap(),
        "ck": nc.dram_tensor("ck", [32], F32, kind="ExternalInput").ap(),
        "sidx": nc.dram_tensor("sidx", [IT_M, S], I32, kind="ExternalInput").ap(),
        "sir": nc.dram_tensor("sir", [IT_M * IT_R, NCON], I32, kind="ExternalInput").ap(),
        "cst": nc.dram_tensor("cst", [576], F32, kind="ExternalInput").ap(),
        "out": nc.dram_tensor("out", [2], F32, kind="ExternalOutput").ap(),
    }
    emit_core(nc, io)
    nc.finalize()
    return nc


def prep_core_inputs(inputs, b):
    f32 = np.float32
    kps0 = np.asarray(inputs["kps0"], f32)[b]      # [2,NK]
    kps1 = np.asarray(inputs["kps1"], f32)[b]
    d0 = np.asarray(inputs["depth0"], f32)[b]      # [1,NK]
    d1 = np.asarray(inputs["depth1"], f32)[b]
